# revision 1
# baseline (speedup 1.0000x reference)
"""Depthwise causal conv1d kernel for Trainium2 (8 NeuronCores, SPMD).

Problem: x [B=8, T=4096, C=512] f32, weight [C=512, K=4] f32.
out[b, t, c] = sum_k weight[c, k] * x[b, t - 3 + k, c]   (causal, zero-pad)

Strategy:
  - Data-parallel over batch: core b handles x[b].
  - Host-side layout: each core's input is channels-first x[b].T padded
    with K-1 = 3 leading zeros along time -> [C=512, T+3=4099], cast to
    fp16, so the device kernel sees contiguous time on the free axis and
    channels on partitions. fp16 halves the input traffic (the kernel is
    HBM-bound) and its 11-bit mantissa keeps the conv's error at the
    ~2^-11 level; accumulation stays fp32 in PSUM.
  - Device: an 8 KB fp16 weight-column table lands first; GpSimd expands
    it into 16 diag(weight[:, k]) [128x128] stationary matrices while the
    x chunks stream in. For each 128-channel chunk the 4-tap conv is 4
    accumulating TensorE matmuls (stationary diag, moving = shifted x
    views) at full single-pass PE rate, paired into 2-bank PSUM tiles.
    PSUM results are cast to fp16 during the PSUM->SBUF copy (ScalarE
    even chunks, VectorE odd — so the last chunk's tail copy is on the
    faster engine) and shipped as soon as each half-chunk's copies land;
    the host upcasts to fp32. ~8.4 MB HBM traffic per core; the kernel
    ends PE-bound at ~29 us of matmul inside a ~46.5 us exec window.
"""

import numpy as np

B, T, C, K = 8, 4096, 512, 4
P = 128  # partitions
NCHUNK = C // P  # 4 channel chunks
TJ = 512  # time-tile (free dim) per matmul; one PSUM bank
NJ = T // TJ  # 8 time tiles per chunk
TP = T + K - 1  # padded time = 4099
NW = NCHUNK * K  # 16 (chunk, tap) pairs

_compiled = None


def _build():
    import concourse.bacc as bacc
    import concourse.bass as bass
    import concourse.mybir as mybir
    from concourse.tile import TileContext

    f32 = mybir.dt.float32
    f16 = mybir.dt.float16
    nc = bacc.Bacc(enable_partition_id=False)

    wt_d = nc.declare_dram_parameter("wt", [P, NW], f16, isOutput=False)
    xw_d = nc.declare_dram_parameter("xw", [P, NCHUNK * TP], f16, isOutput=False)
    out_d = nc.declare_dram_parameter("out", [C, T], f16, isOutput=True)

    with TileContext(nc) as tc:
        with (
            tc.tile_pool(name="xpool", bufs=1) as xpool,
            tc.tile_pool(name="wpool", bufs=1) as wpool,
            tc.tile_pool(name="opool", bufs=4) as opool,
            tc.tile_pool(name="ppool", bufs=4, space="PSUM") as ppool,
        ):
            # weight table first: tiny DMA, expansion overlaps the x loads
            wcol = wpool.tile([P, NW], f16, tag="wcol")
            nc.sync.dma_start(out=wcol, in_=wt_d[:, :])
            # chunk0's x split in two so its first time-tiles land sooner
            xts = []
            xt0 = xpool.tile([P, TP], f16, name="xt0", tag="xt0")
            h0 = TJ + K - 1  # 515 cols: j-tile 0 + halo
            h1 = NJ // 2 * TJ + K - 1  # j-tiles 1..3
            nc.sync.dma_start(out=xt0[:, :h0], in_=xw_d[:, 0:h0])
            nc.sync.dma_start(out=xt0[:, h0:h1], in_=xw_d[:, h0:h1])
            nc.sync.dma_start(out=xt0[:, h1:], in_=xw_d[:, h1:TP])
            xts.append(xt0)
            for c in range(1, NCHUNK):
                xt = xpool.tile([P, TP], f16, name=f"xt{c}", tag=f"xt{c}")
                nc.sync.dma_start(out=xt, in_=xw_d[:, c * TP : (c + 1) * TP])
                xts.append(xt)

            # expand wcol into per-chunk diag stationary tiles on GpSimd;
            # separate tiles so chunk0's matmuls wait only on its own 4
            wts = []
            for c in range(NCHUNK):
                wt = wpool.tile([P, K * P], f16, name=f"wd{c}", tag=f"wd{c}")
                for k in range(K):
                    idx = c * K + k
                    wsrc = bass.AP(
                        wcol.tensor, wcol.offset + idx, [[NW, P], [0, P]]
                    )
                    nc.gpsimd.affine_select(
                        out=wt[:, k * P : (k + 1) * P],
                        in_=wsrc,
                        compare_op=mybir.AluOpType.is_equal,
                        fill=0.0,
                        base=0,
                        # iota[p, i] = p - i; == 0 on the diagonal
                        pattern=[[-1, P]],
                        channel_multiplier=1,
                    )
                wts.append(wt)

            TJ2 = 2 * TJ  # j-tile pair: one 2-bank PSUM tile, one copy
            for chunk in range(NCHUNK):
                xv = xts[chunk]
                wt = wts[chunk]
                ot = opool.tile([P, T], f16, tag="ot")
                for jj in range(NJ // 2):
                    pt = ppool.tile([P, TJ2], f32, name="pt", tag="pt")
                    for half in range(2):
                        j = 2 * jj + half
                        for k in range(K):
                            nc.tensor.matmul(
                                pt[:, half * TJ : (half + 1) * TJ],
                                wt[:, k * P : (k + 1) * P],
                                xv[:, j * TJ + k : j * TJ + k + TJ],
                                start=(k == 0),
                                stop=(k == K - 1),
                            )
                    dst = ot[:, jj * TJ2 : (jj + 1) * TJ2]
                    # DVE takes the odd chunks so the LAST chunk's tail
                    # copy is the faster engine (DVE cast-copy ~0.66 us
                    # vs ACT ~1.0 us)
                    if chunk % 2 == 0:
                        nc.scalar.copy(dst, pt)
                    else:
                        nc.vector.tensor_copy(dst, pt)
                    # ship output as soon as its copies land; the last
                    # chunk goes out in pair-pieces to shorten the tail
                    last = chunk == NCHUNK - 1
                    if last and jj >= NJ // 4:
                        lo_c = jj * TJ2
                        nc.sync.dma_start(
                            out=out_d[chunk * P : (chunk + 1) * P, lo_c : lo_c + TJ2],
                            in_=ot[:, lo_c : lo_c + TJ2],
                        )
                    elif jj == NJ // 4 - 1 or jj == NJ // 2 - 1:
                        half_c = 0 if jj < NJ // 4 else NJ // 4 * TJ2
                        nc.sync.dma_start(
                            out=out_d[
                                chunk * P : (chunk + 1) * P,
                                half_c : half_c + NJ // 4 * TJ2,
                            ],
                            in_=ot[:, half_c : half_c + NJ // 4 * TJ2],
                        )

    nc.compile()
    return nc


def _prep_inputs(x: np.ndarray, weight: np.ndarray):
    # wcol[p, chunk*K + k] = weight[chunk*P + p, k]
    wcol = np.ascontiguousarray(
        weight.reshape(NCHUNK, P, K).transpose(1, 0, 2).reshape(P, NW)
    ).astype(np.float16)
    xs = []
    for b in range(B):
        xp = np.zeros((C, TP), dtype=np.float32)
        xp[:, K - 1 :] = x[b].T  # [512, 4099], 3 leading zeros
        xw = np.ascontiguousarray(
            xp.reshape(NCHUNK, P, TP).transpose(1, 0, 2).reshape(P, NCHUNK * TP)
        ).astype(np.float16)
        xs.append(xw)
    return xs, wcol


def _ensure_axon_hooks():
    """This image's antenv package lacks axon_hooks; synthesize it so a
    trace=True / BASS_TRACE run of run_bass_kernel_spmd can profile
    instead of crashing on import."""
    import sys
    import types

    if "antenv.axon_hooks" in sys.modules:
        return
    mod = types.ModuleType("antenv.axon_hooks")
    state = {"hook": None}
    mod.set_axon_ntff_profile_hook = lambda h: state.__setitem__("hook", h)
    mod.get_axon_ntff_profile_hook = lambda: state["hook"]
    sys.modules["antenv.axon_hooks"] = mod
    try:
        if "/root/.axon_site" not in sys.path:
            sys.path.insert(0, "/root/.axon_site")
        from trn_agent_boot.trn_boot import _ntff_profile_via_ctypes

        mod.set_axon_ntff_profile_hook(
            _ntff_profile_via_ctypes("/opt/axon/libaxon_pjrt.so")
        )
    except Exception:
        pass  # hook stays None; concourse degrades to no-trace


def kernel(x: np.ndarray, weight: np.ndarray) -> np.ndarray:
    global _compiled
    _ensure_axon_hooks()
    from concourse import bass_utils

    x = np.ascontiguousarray(x, dtype=np.float32)
    weight = np.ascontiguousarray(weight, dtype=np.float32)

    if _compiled is None:
        _compiled = _build()
    nc = _compiled

    xs, wcol = _prep_inputs(x, weight)
    in_maps = [{"xw": xs[b], "wt": wcol} for b in range(B)]
    res = bass_utils.run_bass_kernel_spmd(nc, in_maps, core_ids=list(range(B)))

    out = np.empty((B, T, C), dtype=np.float32)
    for b in range(B):
        out[b] = np.asarray(res.results[b]["out"]).astype(np.float32).T
    return out



# revision 6
# speedup vs baseline: 1.1891x; 1.1891x over previous
"""Depthwise causal conv1d kernel for Trainium2 (8 NeuronCores, SPMD).

Problem: x [B=8, T=4096, C=512] f32, weight [C=512, K=4] f32.
out[b, t, c] = sum_k weight[c, k] * x[b, t - 3 + k, c]   (causal, zero-pad)

Strategy (v4):
  - Data-parallel over batch: core b handles x[b].
  - Host-side layout: channels-first x[b].T padded with 3 leading zeros
    -> 4 chunk tiles [128, 4099] fp16; fp16 output (host upcasts).
    Diagonal stationary weight tiles are built on the host and DMA'd.
  - PE computes chunks 1,2,3 (4 accumulating diag-matmuls per 512 PSUM
    slice); ACT casts chunks 1,2 PSUM->fp16 and ships them, DVE casts
    chunk 3 (shipped from the idle SP queue).
  - Chunk 0 is elementwise: ACT does the tap-0 product (activation with
    per-partition scale), DVE taps 1-3 (tensor_scalar) + combining adds
    (tensor_tensor), GpSimd one early a23 add + the chunk-0 ship.
  - Few, wide (2048) ops: semaphore count drives a serialized ~40-110ns
    per-semaphore teardown at kernel end, so instruction/sync economy
    matters as much as engine balance.
"""

import numpy as np

B, T, C, K = 8, 4096, 512, 4
P = 128  # partitions
NCHUNK = C // P  # 4 channel chunks
TP = T + K - 1  # padded time = 4099
NW = NCHUNK * K  # 16 weight columns
H = 2048  # op width
E0 = 0  # elementwise chunk
PE_CHUNKS = (1, 2, 3)
HSPLIT = H + K  # 2052: chunk-0 first-half split (halo incl.)

_compiled = None


def _build():
    import concourse.bacc as bacc
    import concourse.mybir as mybir
    from concourse.tile import TileContext

    f32 = mybir.dt.float32
    f16 = mybir.dt.float16
    Alu = mybir.AluOpType
    nc = bacc.Bacc(enable_partition_id=False)

    wtf32_d = nc.declare_dram_parameter("wtf32", [P, NW], f32, isOutput=False)
    wd_d = {
        c: nc.declare_dram_parameter(f"wd{c}", [P, K * P], f16, isOutput=False)
        for c in PE_CHUNKS
    }
    xw_d = nc.declare_dram_parameter("xw", [P, NCHUNK * TP], f16, isOutput=False)
    out_d = nc.declare_dram_parameter("out", [C, T], f16, isOutput=True)

    with TileContext(nc) as tc:
        with (
            tc.tile_pool(name="xpool", bufs=1) as xpool,
            tc.tile_pool(name="wpool", bufs=1) as wpool,
            tc.tile_pool(name="tpool", bufs=1) as tpool,
            tc.tile_pool(name="opool", bufs=1) as opool,
            tc.tile_pool(name="ppool", bufs=2, space="PSUM") as ppool,
        ):
            c1, c2, c3 = PE_CHUNKS

            # ---- SP input stream
            wtf32 = wpool.tile([P, NW], f32, name="wtf32", tag="wtf32")
            nc.sync.dma_start(out=wtf32, in_=wtf32_d[:, :])
            wd = {
                c: wpool.tile([P, K * P], f16, name=f"wd{c}", tag=f"wd{c}")
                for c in PE_CHUNKS
            }
            xt = {
                c: xpool.tile([P, TP], f16, name=f"xt{c}", tag=f"xt{c}")
                for c in range(NCHUNK)
            }

            def load_x(c, lo, hi):
                nc.sync.dma_start(
                    out=xt[c][:, lo:hi], in_=xw_d[:, c * TP + lo : c * TP + hi]
                )

            nc.sync.dma_start(out=wd[c1], in_=wd_d[c1][:, :])
            load_x(c1, 0, TP)
            load_x(E0, 0, HSPLIT)
            nc.sync.dma_start(out=wd[c2], in_=wd_d[c2][:, :])
            nc.sync.dma_start(out=wd[c3], in_=wd_d[c3][:, :])
            load_x(E0, HSPLIT, TP)
            load_x(c2, 0, TP)
            load_x(c3, 0, TP)

            ot = {
                c: opool.tile([P, T], f16, name=f"ot{c}", tag=f"ot{c}")
                for c in range(NCHUNK)
            }

            # ---- PE: one chunk = 2 PSUM tiles x 4 slices x 4 taps
            pts = {}

            def pe_chunk(c):
                for half in range(2):
                    pt = ppool.tile([P, H], f32, name="pt", tag="pt")
                    pts[(c, half)] = pt
                    for s in range(4):
                        base = half * H + s * 512
                        for k in range(K):
                            nc.tensor.matmul(
                                pt[:, s * 512 : (s + 1) * 512],
                                wd[c][:, k * P : (k + 1) * P],
                                xt[c][:, base + k : base + k + 512],
                                start=(k == 0),
                                stop=(k == K - 1),
                            )

            def cp(eng, c, half):
                eng.copy(
                    ot[c][:, half * H : (half + 1) * H], pts[(c, half)]
                ) if eng is nc.scalar else eng.tensor_copy(
                    ot[c][:, half * H : (half + 1) * H], pts[(c, half)]
                )

            def ship(eng, c, lo, width):
                eng.dma_start(
                    out=out_d[c * P : (c + 1) * P, lo : lo + width],
                    in_=ot[c][:, lo : lo + width],
                )

            # elementwise temps for chunk 0
            m0 = tpool.tile([P, T], f16, name="m0", tag="m0")
            m1 = tpool.tile([P, T], f16, name="m1", tag="m1")
            m2 = tpool.tile([P, T], f16, name="m2", tag="m2")
            m3 = tpool.tile([P, T], f16, name="m3", tag="m3")
            a01 = tpool.tile([P, T], f16, name="a01", tag="a01")
            a23 = tpool.tile([P, T], f16, name="a23", tag="a23")

            def wsl(k):
                return wtf32[:, E0 * K + k : E0 * K + k + 1]

            def hsl(h):
                return slice(h * H, (h + 1) * H)

            def mult(dst, k, h):
                nc.vector.tensor_scalar(
                    out=dst[:, hsl(h)],
                    in0=xt[E0][:, h * H + k : h * H + k + H],
                    scalar1=wsl(k),
                    scalar2=None,
                    op0=Alu.mult,
                )

            def add(dst, x, y, h, eng=None):
                (eng or nc.vector).tensor_tensor(
                    out=dst[:, hsl(h)], in0=x[:, hsl(h)], in1=y[:, hsl(h)],
                    op=Alu.add,
                )

            # ---- emission in dataflow order ----
            pe_chunk(c1)

            # ACT: tap-0 products, then chunk-1 copies/ship
            for h in range(2):
                nc.scalar.activation(
                    out=m0[:, hsl(h)],
                    in_=xt[E0][:, h * H : h * H + H],
                    func=mybir.ActivationFunctionType.Copy,
                    scale=wsl(0),
                )
            # DVE: taps 1-3 first half
            mult(m1, 1, 0)
            mult(m2, 2, 0)
            mult(m3, 3, 0)
            # GpSimd: early a23 on first half
            add(a23, m2, m3, 0, eng=nc.gpsimd)
            # DVE: rest of chunk 0
            add(a01, m0, m1, 0)
            mult(m1, 1, 1)
            mult(m2, 2, 1)
            mult(m3, 3, 1)
            add(a01, m0, m1, 1)
            add(a23, m2, m3, 1)
            add(ot[E0], a01, a23, 0)
            add(ot[E0], a01, a23, 1)
            # GpSimd ships chunk 0 from its own queue
            ship(nc.gpsimd, E0, 0, T)

            cp(nc.scalar, c1, 0)
            cp(nc.scalar, c1, 1)
            ship(nc.scalar, c1, 0, T)

            pe_chunk(c2)
            cp(nc.scalar, c2, 0)
            cp(nc.scalar, c2, 1)
            ship(nc.scalar, c2, 0, T)

            pe_chunk(c3)
            cp(nc.vector, c3, 0)
            ship(nc.sync, c3, 0, H)
            cp(nc.vector, c3, 1)
            ship(nc.sync, c3, H, H)

    nc.compile()
    return nc


def _prep_inputs(x: np.ndarray, weight: np.ndarray):
    # wcol[p, chunk*K + k] = weight[chunk*P + p, k]
    wcol = np.ascontiguousarray(
        weight.reshape(NCHUNK, P, K).transpose(1, 0, 2).reshape(P, NW)
    )
    wtf32 = wcol.astype(np.float32)
    wcol16 = wcol.astype(np.float16)
    wds = {}
    rng = np.arange(P)
    for c in PE_CHUNKS:
        wd = np.zeros((P, K * P), dtype=np.float16)
        for k in range(K):
            wd[rng, k * P + rng] = wcol16[:, c * K + k]
        wds[c] = wd
    xs = []
    for b in range(B):
        xp = np.zeros((C, TP), dtype=np.float32)
        xp[:, K - 1 :] = x[b].T  # [512, 4099], 3 leading zeros
        xw = np.ascontiguousarray(
            xp.reshape(NCHUNK, P, TP).transpose(1, 0, 2).reshape(P, NCHUNK * TP)
        ).astype(np.float16)
        xs.append(xw)
    return xs, wtf32, wds


def _ensure_axon_hooks():
    """This image's antenv package lacks axon_hooks; synthesize it so a
    trace=True / BASS_TRACE run of run_bass_kernel_spmd can profile
    instead of crashing on import."""
    import sys
    import types

    if "antenv.axon_hooks" in sys.modules:
        return
    mod = types.ModuleType("antenv.axon_hooks")
    state = {"hook": None}
    mod.set_axon_ntff_profile_hook = lambda h: state.__setitem__("hook", h)
    mod.get_axon_ntff_profile_hook = lambda: state["hook"]
    sys.modules["antenv.axon_hooks"] = mod
    try:
        if "/root/.axon_site" not in sys.path:
            sys.path.insert(0, "/root/.axon_site")
        from trn_agent_boot.trn_boot import _ntff_profile_via_ctypes

        mod.set_axon_ntff_profile_hook(
            _ntff_profile_via_ctypes("/opt/axon/libaxon_pjrt.so")
        )
    except Exception:
        pass  # hook stays None; concourse degrades to no-trace


def _in_maps(x, weight):
    xs, wtf32, wds = _prep_inputs(x, weight)
    return [
        {
            "xw": xs[b],
            "wtf32": wtf32,
            **{f"wd{c}": wds[c] for c in PE_CHUNKS},
        }
        for b in range(B)
    ]


def kernel(x: np.ndarray, weight: np.ndarray) -> np.ndarray:
    global _compiled
    _ensure_axon_hooks()
    from concourse import bass_utils

    x = np.ascontiguousarray(x, dtype=np.float32)
    weight = np.ascontiguousarray(weight, dtype=np.float32)

    if _compiled is None:
        _compiled = _build()
    nc = _compiled

    res = bass_utils.run_bass_kernel_spmd(
        nc, _in_maps(x, weight), core_ids=list(range(B))
    )

    out = np.empty((B, T, C), dtype=np.float32)
    for b in range(B):
        out[b] = np.asarray(res.results[b]["out"]).astype(np.float32).T
    return out
